# revision 16
# baseline (speedup 1.0000x reference)
"""BFP (block floating point) quantize-dequantize kernel for Trainium2.

Math (per block of 8 along the last dim, zero-padded to a multiple of 8):
    maxabs = max(|x_block|)
    e      = floor(log2(maxabs))            (IEEE unbiased exponent)
    step   = 2^(e-6)
    out    = clip(round_half_even(x/step), -128, 127) * step

Implemented exactly with float/int bit tricks (no division, no round op):
    rstep   = 2^(6-e)    from exponent-field bit arithmetic
    negstep = -2^(e-6)
    y = x * rstep                                    (exact: power-of-2 scale)
    t = fl(y + 12582912.0)                           (RNE round onto int grid)
    r = relu(12583039.0 - t)  == 127 - clip(q, ., 127)
    out = (r - 127) * negstep == clip(q) * step
The lower clip at -128 never binds (|y| < 128 strictly).
All-zero blocks come out as exact 0 with no special casing.

Sharding: rows 8192 -> 1024 per core across 8 NeuronCores, no communication.
"""

import numpy as np

import concourse.bass as bass
import concourse.bacc as bacc
import concourse.tile as tile
from concourse import mybir
from concourse.bass_utils import run_bass_kernel_spmd

# Problem shape (hardcoded per contract: kernel.py is self-contained).
N_ROWS = 8192
N_COLS = 12284
N_CORES = 8
ROWS_PER_CORE = N_ROWS // N_CORES  # 1024
P = 128  # SBUF partitions
ROW_TILES = ROWS_PER_CORE // P  # 8

# Column tiling: 4096 + 4096 + 4092; the last tile is padded on-chip to 4096
# with zeros so every tile is uniform 512 blocks of 8.
COL_TILES = [(0, 4096), (4096, 4096), (8192, 4092)]
W_ALLOC = 4096
NBLK = W_ALLOC // 8  # 512

MAGIC = 12582912.0  # 1.5 * 2^23
MAGIC_HI = 12583039.0  # MAGIC + 127
EXP_MASK = 0x7F800000
SIGN_BIT = -0x80000000  # int32 bit pattern 0x80000000


def _build_kernel(reps=1):
    # Bacc (not raw Bass): its compile() pass legalizes multi-wait sync_info
    # into EventSemaphore chains (TPB instructions encode only 1 sem wait).
    # reps>1 unrolls the whole kernel body for benchmarking (differencing
    # two rep counts cancels host/dispatch overhead).
    nc = bacc.Bacc("TRN2", target_bir_lowering=False, debug=False, num_devices=N_CORES)
    f32 = mybir.dt.float32
    i32 = mybir.dt.int32

    x_d = nc.declare_dram_parameter("x", [ROWS_PER_CORE, N_COLS], f32, isOutput=False)
    o_d = nc.declare_dram_parameter("out", [ROWS_PER_CORE, N_COLS], f32, isOutput=True)

    with tile.TileContext(nc) as tc:
        with (
            tc.tile_pool(name="xp", bufs=3) as xp,
            tc.tile_pool(name="yp", bufs=2) as yp,
            tc.tile_pool(name="tp", bufs=2) as tp,
            tc.tile_pool(name="rp", bufs=2) as rp,
            tc.tile_pool(name="op", bufs=2) as op,
            tc.tile_pool(name="blk", bufs=3) as blk,
            tc.tile_pool(name="singles", bufs=1) as singles,
        ):
            bias_hi = singles.tile([P, 1], f32)
            nc.vector.memset(bias_hi[:], MAGIC_HI)
            for rt in range(ROW_TILES * reps):
                r0 = (rt % ROW_TILES) * P
                for c0, w in COL_TILES:
                    xt = xp.tile([P, W_ALLOC], f32, tag="x")
                    if w < W_ALLOC:
                        nc.vector.memset(xt[:, w:], 0.0)
                    nc.sync.dma_start(xt[:, :w], x_d[r0 : r0 + P, c0 : c0 + w])

                    # block abs-max -> m [P, NBLK]
                    m = blk.tile([P, NBLK], f32, tag="m")
                    nc.vector.tensor_reduce(
                        m[:],
                        xt[:].rearrange("p (b k) -> p b k", k=8),
                        axis=mybir.AxisListType.X,
                        op=mybir.AluOpType.max,
                        apply_absolute_value=True,
                    )
                    mi = m[:].bitcast(i32)

                    # E = biased exponent of maxabs, clamped to >= 26 so the
                    # rstep bits below never overflow int32 (all-zero blocks).
                    ecl = blk.tile([P, NBLK], f32, tag="ecl")
                    nc.vector.tensor_scalar(
                        ecl[:].bitcast(i32),
                        mi,
                        23,
                        None,
                        op0=mybir.AluOpType.logical_shift_right,
                    )
                    nc.vector.tensor_scalar(
                        ecl[:].bitcast(i32),
                        ecl[:].bitcast(i32),
                        26,
                        None,
                        op0=mybir.AluOpType.max,
                    )

                    # rstep = 2^(6-e): bits = (133-e)<<23 = (E-260) * -2^23
                    rs = blk.tile([P, NBLK], f32, tag="rs")
                    nc.vector.tensor_scalar(
                        rs[:].bitcast(i32),
                        ecl[:].bitcast(i32),
                        260,
                        -8388608,
                        op0=mybir.AluOpType.subtract,
                        op1=mybir.AluOpType.mult,
                    )

                    # negstep = -(2^(e-6)): bits(int32) = (E-262) * 2^23
                    ns = blk.tile([P, NBLK], f32, tag="ns")
                    nc.vector.tensor_scalar(
                        ns[:].bitcast(i32),
                        ecl[:].bitcast(i32),
                        262,
                        8388608,
                        op0=mybir.AluOpType.subtract,
                        op1=mybir.AluOpType.mult,
                    )

                    # y = x * rstep  (broadcast rstep over each block of 8)
                    yt = yp.tile([P, W_ALLOC], f32, tag="y")
                    rs_b = bass.AP(
                        tensor=rs[:].tensor,
                        offset=rs[:].offset,
                        ap=[rs[:].ap[0], rs[:].ap[1], [0, 8]],
                    )
                    nc.gpsimd.tensor_tensor(
                        yt[:].rearrange("p (b k) -> p b k", k=8),
                        xt[:].rearrange("p (b k) -> p b k", k=8),
                        rs_b,
                        op=mybir.AluOpType.mult,
                    )

                    # t = fl(y + MAGIC)  (RNE onto integer grid)
                    tt = tp.tile([P, W_ALLOC], f32, tag="t")
                    nc.scalar.activation(
                        tt[:], yt[:], mybir.ActivationFunctionType.Copy, bias=MAGIC
                    )

                    # r = relu(MAGIC_HI - t) = 127 - clip(q)
                    rt_t = rp.tile([P, W_ALLOC], f32, tag="r")
                    nc.scalar.activation(
                        rt_t[:],
                        tt[:],
                        mybir.ActivationFunctionType.Relu,
                        bias=bias_hi[:],
                        scale=-1.0,
                    )

                    # out = (r - 127) * negstep
                    ot = op.tile([P, W_ALLOC], f32, tag="o")
                    ns_b = bass.AP(
                        tensor=ns[:].tensor,
                        offset=ns[:].offset,
                        ap=[ns[:].ap[0], ns[:].ap[1], [0, 8]],
                    )
                    nc.vector.scalar_tensor_tensor(
                        ot[:].rearrange("p (b k) -> p b k", k=8),
                        rt_t[:].rearrange("p (b k) -> p b k", k=8),
                        127.0,
                        ns_b,
                        op0=mybir.AluOpType.subtract,
                        op1=mybir.AluOpType.mult,
                    )

                    nc.sync.dma_start(o_d[r0 : r0 + P, c0 : c0 + w], ot[:, :w])

    nc.compile()
    return nc


_NC_CACHE = None


def kernel(x: np.ndarray) -> np.ndarray:
    global _NC_CACHE
    assert x.shape == (N_ROWS, N_COLS) and x.dtype == np.float32
    if _NC_CACHE is None:
        _NC_CACHE = _build_kernel()
    nc = _NC_CACHE
    in_maps = [
        {"x": np.ascontiguousarray(x[c * ROWS_PER_CORE : (c + 1) * ROWS_PER_CORE])}
        for c in range(N_CORES)
    ]
    res = run_bass_kernel_spmd(nc, in_maps, list(range(N_CORES))).results
    return np.concatenate([res[c]["out"] for c in range(N_CORES)], axis=0)


# revision 19
# speedup vs baseline: 2731.1192x; 2731.1192x over previous
"""BFP (block floating point) quantize-dequantize kernel for Trainium2.

Math (per block of 8 along the last dim, zero-padded to a multiple of 8):
    maxabs = max(|x_block|)
    e      = floor(log2(maxabs))            (IEEE unbiased exponent)
    step   = 2^(e-6)
    out    = clip(round_half_even(x/step), -128, 127) * step

Implemented exactly with float/int bit tricks (no division, no round op):
    rstep   = 2^(6-e)    from exponent-field bit arithmetic
    negstep = -2^(e-6)
    y = x * rstep                                    (exact: power-of-2 scale)
    t = fl(y + 12582912.0)                           (RNE round onto int grid)
    r = relu(12583039.0 - t)  == 127 - clip(q, ., 127)
    out = (r - 127) * negstep == clip(q) * step
The lower clip at -128 never binds (|y| < 128 strictly).
All-zero blocks come out as exact 0 with no special casing.

Sharding: rows 8192 -> 1024 per core across 8 NeuronCores, no communication.
"""

import numpy as np

import concourse.bass as bass
import concourse.bacc as bacc
import concourse.tile as tile
from concourse import mybir
from concourse.bass_utils import run_bass_kernel_spmd

# Problem shape (hardcoded per contract: kernel.py is self-contained).
N_ROWS = 8192
N_COLS = 12284
N_CORES = 8
ROWS_PER_CORE = N_ROWS // N_CORES  # 1024
P = 128  # SBUF partitions
ROW_TILES = ROWS_PER_CORE // P  # 8

# Column tiling: 4096 + 4096 + 4092; the last tile is padded on-chip to 4096
# with zeros so every tile is uniform 512 blocks of 8.
COL_TILES = [(0, 4096), (4096, 4096), (8192, 4092)]
W_ALLOC = 4096
NBLK = W_ALLOC // 8  # 512

MAGIC = 12582912.0  # 1.5 * 2^23
MAGIC_HI = 12583039.0  # MAGIC + 127
EXP_MASK = 0x7F800000
SIGN_BIT = -0x80000000  # int32 bit pattern 0x80000000


def _build_kernel(reps=1, loop_reps=0):
    # Bacc (not raw Bass): its compile() pass legalizes multi-wait sync_info
    # into EventSemaphore chains (TPB instructions encode only 1 sem wait).
    # reps>1 unrolls the whole kernel body; loop_reps>0 additionally wraps
    # it in a hardware For_i loop — both only for benchmarking (differencing
    # two rep counts cancels host/dispatch overhead).
    nc = bacc.Bacc("TRN2", target_bir_lowering=False, debug=False, num_devices=N_CORES)
    f32 = mybir.dt.float32
    i32 = mybir.dt.int32

    x_d = nc.declare_dram_parameter("x", [ROWS_PER_CORE, N_COLS], f32, isOutput=False)
    o_d = nc.declare_dram_parameter("out", [ROWS_PER_CORE, N_COLS], f32, isOutput=True)

    with tile.TileContext(nc) as tc:
        with (
            tc.tile_pool(name="xp", bufs=3) as xp,
            tc.tile_pool(name="yp", bufs=2) as yp,
            tc.tile_pool(name="tp", bufs=2) as tp,
            tc.tile_pool(name="rp", bufs=2) as rp,
            tc.tile_pool(name="op", bufs=2) as op,
            tc.tile_pool(name="blk", bufs=3) as blk,
            tc.tile_pool(name="singles", bufs=1) as singles,
        ):
            bias_hi = singles.tile([P, 1], f32)
            nc.vector.memset(bias_hi[:], MAGIC_HI)

            from contextlib import nullcontext

            loop_cm = tc.For_i(0, loop_reps, 1) if loop_reps else nullcontext()
            with loop_cm:
                _body(nc, tc, x_d, o_d, bias_hi, xp, yp, tp, rp, op, blk, reps)

    nc.compile()
    return nc


def _body(nc, tc, x_d, o_d, bias_hi, xp, yp, tp, rp, op, blk, reps):
    f32 = mybir.dt.float32
    i32 = mybir.dt.int32
    if True:
        if True:
            for rt in range(ROW_TILES * reps):
                r0 = (rt % ROW_TILES) * P
                for c0, w in COL_TILES:
                    xt = xp.tile([P, W_ALLOC], f32, tag="x")
                    if w < W_ALLOC:
                        nc.vector.memset(xt[:, w:], 0.0)
                    nc.sync.dma_start(xt[:, :w], x_d[r0 : r0 + P, c0 : c0 + w])

                    # block abs-max -> m [P, NBLK]
                    m = blk.tile([P, NBLK], f32, tag="m")
                    nc.vector.tensor_reduce(
                        m[:],
                        xt[:].rearrange("p (b k) -> p b k", k=8),
                        axis=mybir.AxisListType.X,
                        op=mybir.AluOpType.max,
                        apply_absolute_value=True,
                    )
                    mi = m[:].bitcast(i32)

                    # E = biased exponent of maxabs, clamped to >= 26 so the
                    # rstep bits below never overflow int32 (all-zero blocks).
                    ecl = blk.tile([P, NBLK], f32, tag="ecl")
                    nc.vector.tensor_scalar(
                        ecl[:].bitcast(i32),
                        mi,
                        23,
                        None,
                        op0=mybir.AluOpType.logical_shift_right,
                    )
                    nc.vector.tensor_scalar(
                        ecl[:].bitcast(i32),
                        ecl[:].bitcast(i32),
                        26,
                        None,
                        op0=mybir.AluOpType.max,
                    )

                    # rstep = 2^(6-e): bits = (133-e)<<23 = (E-260) * -2^23
                    rs = blk.tile([P, NBLK], f32, tag="rs")
                    nc.vector.tensor_scalar(
                        rs[:].bitcast(i32),
                        ecl[:].bitcast(i32),
                        260,
                        -8388608,
                        op0=mybir.AluOpType.subtract,
                        op1=mybir.AluOpType.mult,
                    )

                    # negstep = -(2^(e-6)): bits(int32) = (E-262) * 2^23
                    ns = blk.tile([P, NBLK], f32, tag="ns")
                    nc.vector.tensor_scalar(
                        ns[:].bitcast(i32),
                        ecl[:].bitcast(i32),
                        262,
                        8388608,
                        op0=mybir.AluOpType.subtract,
                        op1=mybir.AluOpType.mult,
                    )

                    # y = x * rstep  (broadcast rstep over each block of 8)
                    yt = yp.tile([P, W_ALLOC], f32, tag="y")
                    rs_b = bass.AP(
                        tensor=rs[:].tensor,
                        offset=rs[:].offset,
                        ap=[rs[:].ap[0], rs[:].ap[1], [0, 8]],
                    )
                    nc.gpsimd.tensor_tensor(
                        yt[:].rearrange("p (b k) -> p b k", k=8),
                        xt[:].rearrange("p (b k) -> p b k", k=8),
                        rs_b,
                        op=mybir.AluOpType.mult,
                    )

                    # t = fl(y + MAGIC)  (RNE onto integer grid)
                    tt = tp.tile([P, W_ALLOC], f32, tag="t")
                    nc.scalar.activation(
                        tt[:], yt[:], mybir.ActivationFunctionType.Copy, bias=MAGIC
                    )

                    # r = relu(MAGIC_HI - t) = 127 - clip(q)
                    rt_t = rp.tile([P, W_ALLOC], f32, tag="r")
                    nc.scalar.activation(
                        rt_t[:],
                        tt[:],
                        mybir.ActivationFunctionType.Relu,
                        bias=bias_hi[:],
                        scale=-1.0,
                    )

                    # out = (r - 127) * negstep
                    ot = op.tile([P, W_ALLOC], f32, tag="o")
                    ns_b = bass.AP(
                        tensor=ns[:].tensor,
                        offset=ns[:].offset,
                        ap=[ns[:].ap[0], ns[:].ap[1], [0, 8]],
                    )
                    nc.vector.scalar_tensor_tensor(
                        ot[:].rearrange("p (b k) -> p b k", k=8),
                        rt_t[:].rearrange("p (b k) -> p b k", k=8),
                        127.0,
                        ns_b,
                        op0=mybir.AluOpType.subtract,
                        op1=mybir.AluOpType.mult,
                    )

                    nc.sync.dma_start(o_d[r0 : r0 + P, c0 : c0 + w], ot[:, :w])


_NC_CACHE = None


def kernel(x: np.ndarray) -> np.ndarray:
    global _NC_CACHE
    assert x.shape == (N_ROWS, N_COLS) and x.dtype == np.float32
    if _NC_CACHE is None:
        _NC_CACHE = _build_kernel()
    nc = _NC_CACHE
    in_maps = [
        {"x": np.ascontiguousarray(x[c * ROWS_PER_CORE : (c + 1) * ROWS_PER_CORE])}
        for c in range(N_CORES)
    ]
    res = run_bass_kernel_spmd(nc, in_maps, list(range(N_CORES))).results
    return np.concatenate([res[c]["out"] for c in range(N_CORES)], axis=0)


# revision 24
# speedup vs baseline: 3178.7642x; 1.1639x over previous
"""BFP (block floating point) quantize-dequantize kernel for Trainium2.

Math (per block of 8 along the last dim, zero-padded to a multiple of 8):
    maxabs = max(|x_block|)
    e      = floor(log2(maxabs))            (IEEE unbiased exponent)
    step   = 2^(e-6)
    out    = clip(round_half_even(x/step), -128, 127) * step

Implemented exactly with float/int bit tricks (no division, no round op):
    rstep   = 2^(6-e)    from exponent-field bit arithmetic
    negstep = -2^(e-6)
    y = x * rstep                                    (exact: power-of-2 scale)
    t = fl(y + 12582912.0)                           (RNE round onto int grid)
    r = relu(12583039.0 - t)  == 127 - clip(q, ., 127)
    out = (r - 127) * negstep == clip(q) * step
The lower clip at -128 never binds (|y| < 128 strictly).
All-zero blocks come out as exact 0 with no special casing.

Sharding: rows 8192 -> 1024 per core across 8 NeuronCores, no communication.
"""

import numpy as np

import concourse.bass as bass
import concourse.bacc as bacc
import concourse.tile as tile
from concourse import mybir
from concourse.bass_utils import run_bass_kernel_spmd

# Problem shape (hardcoded per contract: kernel.py is self-contained).
N_ROWS = 8192
N_COLS = 12284
N_CORES = 8
ROWS_PER_CORE = N_ROWS // N_CORES  # 1024
P = 128  # SBUF partitions
ROW_TILES = ROWS_PER_CORE // P  # 8

# Column tiling: uniform W_ALLOC-wide tiles; the ragged last tile is padded
# on-chip with zeros so every tile is a whole number of 8-blocks.
W_ALLOC = 2048
COL_TILES = []
for _c0 in range(0, N_COLS, W_ALLOC):
    COL_TILES.append((_c0, min(W_ALLOC, N_COLS - _c0)))
NBLK = W_ALLOC // 8
BUFS = {"x": 4, "y": 3, "t": 3, "r": 3, "o": 3, "blk": 4}

MAGIC = 12582912.0  # 1.5 * 2^23
MAGIC_HI = 12583039.0  # MAGIC + 127
EXP_MASK = 0x7F800000
SIGN_BIT = -0x80000000  # int32 bit pattern 0x80000000


def _build_kernel(reps=1, loop_reps=0):
    # Bacc (not raw Bass): its compile() pass legalizes multi-wait sync_info
    # into EventSemaphore chains (TPB instructions encode only 1 sem wait).
    # reps>1 unrolls the whole kernel body; loop_reps>0 additionally wraps
    # it in a hardware For_i loop — both only for benchmarking (differencing
    # two rep counts cancels host/dispatch overhead).
    nc = bacc.Bacc("TRN2", target_bir_lowering=False, debug=False, num_devices=N_CORES)
    f32 = mybir.dt.float32
    i32 = mybir.dt.int32

    x_d = nc.declare_dram_parameter("x", [ROWS_PER_CORE, N_COLS], f32, isOutput=False)
    o_d = nc.declare_dram_parameter("out", [ROWS_PER_CORE, N_COLS], f32, isOutput=True)

    with tile.TileContext(nc) as tc:
        with (
            tc.tile_pool(name="xp", bufs=BUFS["x"]) as xp,
            tc.tile_pool(name="yp", bufs=BUFS["y"]) as yp,
            tc.tile_pool(name="tp", bufs=BUFS["t"]) as tp,
            tc.tile_pool(name="rp", bufs=BUFS["r"]) as rp,
            tc.tile_pool(name="op", bufs=BUFS["o"]) as op,
            tc.tile_pool(name="blk", bufs=BUFS["blk"]) as blk,
            tc.tile_pool(name="singles", bufs=1) as singles,
        ):
            bias_hi = singles.tile([P, 1], f32)
            nc.vector.memset(bias_hi[:], MAGIC_HI)

            from contextlib import nullcontext

            loop_cm = tc.For_i(0, loop_reps, 1) if loop_reps else nullcontext()
            with loop_cm:
                _body(nc, tc, x_d, o_d, bias_hi, xp, yp, tp, rp, op, blk, reps)

    nc.compile()
    return nc


def _body(nc, tc, x_d, o_d, bias_hi, xp, yp, tp, rp, op, blk, reps):
    f32 = mybir.dt.float32
    i32 = mybir.dt.int32

    def stage_front(r0, c0, w):
        """DMA-in -> abs-max -> per-block steps -> Pool mult -> ACT x2."""
        xt = xp.tile([P, W_ALLOC], f32, tag="x")
        if w < W_ALLOC:
            nc.vector.memset(xt[:, w:], 0.0)
        nc.sync.dma_start(xt[:, :w], x_d[r0 : r0 + P, c0 : c0 + w])

        # block abs-max -> m [P, NBLK]
        m = blk.tile([P, NBLK], f32, tag="m")
        nc.vector.tensor_reduce(
            m[:],
            xt[:].rearrange("p (b k) -> p b k", k=8),
            axis=mybir.AxisListType.X,
            op=mybir.AluOpType.max,
            apply_absolute_value=True,
        )

        # E = biased exponent of maxabs, clamped >= 26 so rstep bits
        # never overflow int32 (all-zero blocks). High priority: these
        # tiny ops gate the Pool mult — don't let the scheduler slot
        # later tiles' reduces ahead of them on the DVE.
        with tc.high_priority():
            ecl = blk.tile([P, NBLK], f32, tag="ecl")
            nc.vector.tensor_scalar(
                ecl[:].bitcast(i32), m[:].bitcast(i32), 23, None,
                op0=mybir.AluOpType.logical_shift_right,
            )
            nc.vector.tensor_scalar(
                ecl[:].bitcast(i32), ecl[:].bitcast(i32), 26, None,
                op0=mybir.AluOpType.max,
            )
            # rstep = 2^(6-e): bits = (133-e)<<23 = (E-260) * -2^23
            rs = blk.tile([P, NBLK], f32, tag="rs")
            nc.vector.tensor_scalar(
                rs[:].bitcast(i32), ecl[:].bitcast(i32), 260, -8388608,
                op0=mybir.AluOpType.subtract, op1=mybir.AluOpType.mult,
            )
            # negstep = -(2^(e-6)): bits(int32) = (E-262) * 2^23
            ns = blk.tile([P, NBLK], f32, tag="ns")
            nc.vector.tensor_scalar(
                ns[:].bitcast(i32), ecl[:].bitcast(i32), 262, 8388608,
                op0=mybir.AluOpType.subtract, op1=mybir.AluOpType.mult,
            )

        # y = x * rstep  (broadcast rstep over each block of 8)
        yt = yp.tile([P, W_ALLOC], f32, tag="y")
        rs_b = bass.AP(
            tensor=rs[:].tensor, offset=rs[:].offset,
            ap=[rs[:].ap[0], rs[:].ap[1], [0, 8]],
        )
        nc.gpsimd.tensor_tensor(
            yt[:].rearrange("p (b k) -> p b k", k=8),
            xt[:].rearrange("p (b k) -> p b k", k=8),
            rs_b,
            op=mybir.AluOpType.mult,
        )

        # t = fl(y + MAGIC): the RNE rounding onto the integer grid
        tt = tp.tile([P, W_ALLOC], f32, tag="t")
        nc.scalar.activation(
            tt[:], yt[:], mybir.ActivationFunctionType.Copy, bias=MAGIC
        )
        # r = relu(MAGIC_HI - t) = 127 - clip(q)
        rt_t = rp.tile([P, W_ALLOC], f32, tag="r")
        nc.scalar.activation(
            rt_t[:], tt[:], mybir.ActivationFunctionType.Relu,
            bias=bias_hi[:], scale=-1.0,
        )
        return (rt_t, ns, r0, c0, w)

    def stage_back(ctx):
        """out = (r - 127) * negstep -> DMA-out. Emitted one tile late so
        the DVE never stalls waiting on this tile's ACT output."""
        rt_t, ns, r0, c0, w = ctx
        ot = op.tile([P, W_ALLOC], f32, tag="o")
        ns_b = bass.AP(
            tensor=ns[:].tensor, offset=ns[:].offset,
            ap=[ns[:].ap[0], ns[:].ap[1], [0, 8]],
        )
        nc.vector.scalar_tensor_tensor(
            ot[:].rearrange("p (b k) -> p b k", k=8),
            rt_t[:].rearrange("p (b k) -> p b k", k=8),
            127.0,
            ns_b,
            op0=mybir.AluOpType.subtract,
            op1=mybir.AluOpType.mult,
        )
        # Stores go through the Activation-engine HWDGE queues so they never
        # head-of-line block input loads (SP HWDGE queues).
        nc.scalar.dma_start(o_d[r0 : r0 + P, c0 : c0 + w], ot[:, :w])

    pending = None
    for rt in range(ROW_TILES * reps):
        r0 = (rt % ROW_TILES) * P
        for c0, w in COL_TILES:
            ctx = stage_front(r0, c0, w)
            if pending is not None:
                stage_back(pending)
            pending = ctx
    if pending is not None:
        stage_back(pending)


_NC_CACHE = None


def kernel(x: np.ndarray) -> np.ndarray:
    global _NC_CACHE
    assert x.shape == (N_ROWS, N_COLS) and x.dtype == np.float32
    if _NC_CACHE is None:
        _NC_CACHE = _build_kernel()
    nc = _NC_CACHE
    in_maps = [
        {"x": np.ascontiguousarray(x[c * ROWS_PER_CORE : (c + 1) * ROWS_PER_CORE])}
        for c in range(N_CORES)
    ]
    res = run_bass_kernel_spmd(nc, in_maps, list(range(N_CORES))).results
    return np.concatenate([res[c]["out"] for c in range(N_CORES)], axis=0)
